# revision 26
# baseline (speedup 1.0000x reference)
"""Trainium2 Bass kernel for nn_Encoder_75634374083175.

Single transformer encoder layer: MHA (12 heads, D=768) + LayerNorm + FFN(3072,
exact GELU) + LayerNorm, on [4, 2048, 768] fp32 input with an int key-padding
mask.

Sharding: 8-way data parallel over (batch, half-sequence). Core c handles batch
c//2 and query rows [half*1024, (half+1)*1024). Each core re-projects K/V for
its full batch (duplicated across the core pair) so there are no collectives.

Device layout: all activations are kept "feature-major" ([feature, token] with
features on SBUF partitions) so every matmul uses the weight as the stationary
operand and the activation as the moving operand. Scores are computed directly
transposed (S_T[k_tok, q_tok]), so the softmax exp's additive mask is a
per-partition ACT bias, and the unnormalized attention matrix feeds the context
matmul with no transposes anywhere. Softmax denominators come from a ones
column appended to V (fused into the context matmul); reciprocals and rsqrt
are computed as exp(-ln(x)) on the ACT engine (single table set).
"""

import sys

sys.path.insert(0, "/opt/trn_rl_repo")

import numpy as np
import ml_dtypes

import concourse.bass as bass
import concourse.tile as tile
from concourse import bacc
from concourse import mybir
from contextlib import ExitStack

F32 = mybir.dt.float32
F32R = mybir.dt.float32r
BF16 = mybir.dt.bfloat16
AF = mybir.ActivationFunctionType
ALU = mybir.AluOpType

D = 768          # hidden
H = 12           # heads
HD = 64          # head dim
F = 3072         # ffn hidden
S = 2048         # full sequence per batch
Q = 1024         # query rows per core
P = 128          # partitions
DT = D // P      # 6 feature tiles
ST = S // P      # 16 key-token tiles
FT = F // P      # 24 ffn feature tiles
NCH = 512        # matmul moving free-dim chunk
QC = Q // NCH    # 2 query chunks
KC = S // NCH    # 4 key-token chunks
LN_EPS = 1e-5
MASK_NEG = -30000.0

N_CORES = 8


def _emit(nc, tc, t):
    """Emit the whole encoder layer under TileContext `tc`.

    `t` is a dict of DRAM APs (see build())."""
    es = ExitStack()

    xT_r = t["xT"].rearrange("(t p) m -> t p m", p=P)        # [6,128,2048] f32
    wqT_r = t["wqT"].rearrange("(t p) m -> t p m", p=P)      # [6,128,768] bf16
    wkT_r = t["wkT"].rearrange("(t p) m -> t p m", p=P)
    wvT_r = t["wvT"].rearrange("(t p) m -> t p m", p=P)
    woT_r = t["woT"].rearrange("(t p) m -> t p m", p=P)
    w1T_r = t["w1T"].rearrange("(t p) m -> t p m", p=P)      # [6,128,3072]
    w2T_r = t["w2T"].rearrange("(t p) m -> t p m", p=P)      # [24,128,768]
    outT_r = t["outT"].rearrange("(t p) m -> t p m", p=P)    # [6,128,1024] f32

    # ---------------- long-lived pools ----------------
    consts = es.enter_context(tc.tile_pool(name="consts", bufs=1))
    work = es.enter_context(tc.tile_pool(name="work", bufs=4))
    stat = es.enter_context(tc.tile_pool(name="stat", bufs=2))
    outp = es.enter_context(tc.tile_pool(name="outp", bufs=4))

    mask_sb = consts.tile([P, ST], F32, name="mask", tag="mask")
    nc.sync.dma_start(out=mask_sb[:], in_=t["maskc"][:])
    lng_sb = consts.tile([P, DT], F32, name="lng", tag="lng")
    nc.sync.dma_start(out=lng_sb[:], in_=t["lng"][:])
    lnb_sb = consts.tile([P, DT], F32, name="lnb", tag="lnb")
    nc.sync.dma_start(out=lnb_sb[:], in_=t["lnb"][:])
    ones_f = consts.tile([P, P], F32, name="ones_f", tag="ones_f")
    nc.vector.memset(ones_f[:], 1.0)
    ones_bf = consts.tile([P, 1], BF16, name="ones_bf", tag="ones_bf")
    nc.vector.memset(ones_bf[:], 1.0)
    ones_bfr = consts.tile([1, P], BF16, name="ones_bfr", tag="ones_bfr")
    nc.vector.memset(ones_bfr[:], 1.0)
    eps1 = consts.tile([1, 1], F32, name="eps1", tag="eps1")
    nc.vector.memset(eps1[:], LN_EPS)

    def pe_touch(ap_row, pool, tag="mm"):
        """Tiny matmul reading ap_row ([1,>=8] slice) so PE observes its
        producer's semaphore; keeps DMA waits off real matmuls."""
        tps = pool.tile([1, 8], F32, name="tch", tag=tag)
        nc.tensor.matmul(tps[:], ap_row[0:1, 0:1], ap_row[0:1, 0:8],
                         start=True, stop=True)

    def bcast_row(src_row, rows, ps_pool, tag):
        """[1,N] f32 -> psum [rows,N] f32 broadcast, exact via hi+lo bf16."""
        srow = work.tile([1, NCH], F32, name="brow", tag="brow", bufs=2)
        nc.vector.tensor_copy(srow[:], src_row)
        hi = work.tile([1, NCH], BF16, name="bchi", tag="bchi", bufs=2)
        nc.vector.tensor_copy(hi[:], srow[:])
        lo32 = work.tile([1, NCH], F32, name="bclo32", tag="bclo32", bufs=2)
        nc.vector.tensor_tensor(lo32[:], srow[:], hi[:], ALU.subtract)
        lo = work.tile([1, NCH], BF16, name="bclo", tag="bclo", bufs=2)
        nc.vector.tensor_copy(lo[:], lo32[:])
        bc = ps_pool.tile([rows, NCH], F32, name=tag, tag=tag)
        nc.tensor.matmul(bc[:], ones_bfr[0:1, 0:rows], hi[:],
                         start=True, stop=False)
        nc.tensor.matmul(bc[:], ones_bfr[0:1, 0:rows], lo[:],
                         start=False, stop=True)
        return bc

    # ---------------- phase 1: projections ----------------
    es_kqv = ExitStack()
    kqv = es_kqv.enter_context(tc.tile_pool(name="kqv", bufs=1))
    K_sb = [kqv.tile([P, S], BF16, name=f"K{i}", tag=f"K{i}") for i in range(DT)]
    Q_sb = [kqv.tile([P, Q], BF16, name=f"Q{i}", tag=f"Q{i}") for i in range(DT)]
    # V token-major with a ones column per head: [k_tok, 12*(64+1)]
    V_sb = [kqv.tile([P, H * (HD + 1)], BF16, name=f"V{i}", tag=f"V{i}") for i in range(ST)]

    es_xw = ExitStack()
    es_pp = ExitStack()
    ps_proj = es_pp.enter_context(
        tc.tile_pool(name="ps_proj", bufs=2, space="PSUM"))
    xw = es_xw.enter_context(tc.tile_pool(name="xw", bufs=1))
    wqk = es_pp.enter_context(tc.tile_pool(name="wqk", bufs=1, side="right"))
    # single tiles + single DMAs so consumers wait on few semaphores
    xbf_a = xw.tile([P, DT, S], BF16, name="xbf", tag="xbf")
    wq_a = wqk.tile([P, DT, D], BF16, name="wqa", tag="wqa")
    wk_a = wqk.tile([P, DT, D], BF16, name="wka", tag="wka")
    wv_a = xw.tile([P, DT, D], BF16, name="wva", tag="wva")
    # f32 -> bf16 cast during DMA (SWDGE)
    nc.gpsimd.dma_start(out=xbf_a[:], in_=t["xT"].rearrange("(t p) m -> p t m", p=P))
    nc.sync.dma_start(out=wk_a[:], in_=t["wkT"].rearrange("(t p) m -> p t m", p=P))
    nc.sync.dma_start(out=wv_a[:], in_=t["wvT"].rearrange("(t p) m -> p t m", p=P))
    nc.sync.dma_start(out=wq_a[:], in_=t["wqT"].rearrange("(t p) m -> p t m", p=P))
    pe_touch(xbf_a[:, 0, :], ps_proj)
    pe_touch(wk_a[:, 0, :], ps_proj)
    pe_touch(wv_a[:, 0, :], ps_proj)
    pe_touch(wq_a[:, 0, :], ps_proj)
    xbf = [xbf_a[:, i, :] for i in range(DT)]
    wq_sb = [wq_a[:, i, :] for i in range(DT)]
    wk_sb = [wk_a[:, i, :] for i in range(DT)]
    wv_sb = [wv_a[:, i, :] for i in range(DT)]

    # K_T[f, k_tok] = sum_d WkT[d, f] * xT[d, k_tok]
    for mt in range(DT):
        for ck in range(KC):
            ps = ps_proj.tile([P, NCH], F32, name="mm", tag="mm")
            for kt in range(DT):
                nc.tensor.matmul(
                    ps[:],
                    wk_sb[kt][:, mt * P:(mt + 1) * P],
                    xbf[kt][:, ck * NCH:(ck + 1) * NCH],
                    start=(kt == 0), stop=(kt == DT - 1),
                )
            nc.vector.tensor_copy(K_sb[mt][:, ck * NCH:(ck + 1) * NCH], ps[:])

    def v_proj(tt, pool):
        # V token-major: V[k_tok, f] = sum_d xT[d, k_tok] * WvT[d, f]
        for (c0, c1) in ((0, NCH), (NCH, D)):
            w = c1 - c0
            ps = pool.tile([P, NCH], F32, name="mm", tag="mm")
            for kt in range(DT):
                nc.tensor.matmul(
                    ps[:, 0:w],
                    xbf[kt][:, tt * P:(tt + 1) * P],
                    wv_sb[kt][:, c0:c1],
                    start=(kt == 0), stop=(kt == DT - 1),
                )
            # scatter per head: psum cols are v-features (c0..c1)
            h0 = c0 // HD
            nh = w // HD
            dst = V_sb[tt][:, h0 * (HD + 1):(h0 + nh) * (HD + 1)]
            dst = dst.rearrange("p (h c) -> p h c", c=HD + 1)[:, :, 0:HD]
            src = ps[:, 0:w].rearrange("p (h c) -> p h c", c=HD)
            nc.vector.tensor_copy(dst, src)
        # ones column per head (softmax denominator accumulator)
        oc = V_sb[tt].rearrange("p (h c) -> p h c", c=HD + 1)[:, :, HD:HD + 1]
        nc.vector.memset(oc, 1.0)

    # Q_T[f, q] (queries are the first Q tokens; wqT pre-scaled by 1/8)
    for mt in range(DT):
        for qc in range(QC):
            ps = ps_proj.tile([P, NCH], F32, name="mm", tag="mm")
            for kt in range(DT):
                nc.tensor.matmul(
                    ps[:],
                    wq_sb[kt][:, mt * P:(mt + 1) * P],
                    xbf[kt][:, qc * NCH:(qc + 1) * NCH],
                    start=(kt == 0), stop=(kt == DT - 1),
                )
            nc.vector.tensor_copy(Q_sb[mt][:, qc * NCH:(qc + 1) * NCH], ps[:])

    es_pp.close()

    # ---------------- phase 2: attention ----------------
    es_c = ExitStack()
    cpool = es_c.enter_context(tc.tile_pool(name="cpool", bufs=1, side="right"))
    C_sb = [cpool.tile([P, Q], BF16, name=f"C{i}", tag=f"C{i}") for i in range(DT)]
    wo_a = cpool.tile([P, DT, D], BF16, name="woa", tag="woa")
    xq_a = cpool.tile([P, DT, Q], F32, name="xqa", tag="xqa")
    nc.sync.dma_start(out=wo_a[:], in_=t["woT"].rearrange("(t p) m -> p t m", p=P))
    nc.sync.dma_start(
        out=xq_a[:],
        in_=t["xT"].rearrange("(t p) m -> p t m", p=P)[:, :, 0:Q])

    wo_sb = [wo_a[:, i, :] for i in range(DT)]
    xq_sb = [xq_a[:, i, :] for i in range(DT)]

    es_att = ExitStack()
    ps_v = es_att.enter_context(
        tc.tile_pool(name="ps_v", bufs=1, space="PSUM"))
    ps_smm = es_att.enter_context(
        tc.tile_pool(name="ps_smm", bufs=2, space="PSUM"))
    ps_ctx = es_att.enter_context(tc.tile_pool(name="ps_ctx", bufs=1, space="PSUM", side="right"))

    es_cr = ExitStack()
    crp = es_cr.enter_context(tc.tile_pool(name="crp", bufs=1, side="right"))
    craw = [crp.tile([P, NCH], BF16, name=f"craw{i}", tag=f"craw{i}")
            for i in range(H)]                  # (hp, qc) pair tiles
    # 24 denominator rows in a [128, 6*NCH] grid: row 32*(blk%4), col blk//4
    sums_g = crp.tile([P, 6 * NCH], F32, name="sums", tag="sums")
    nc.vector.memset(sums_g[:], 1.0)

    def sums_slot(blk):
        r = 32 * (blk % 4)
        c = (blk // 4) * NCH
        return sums_g[r:r + 1, c:c + NCH]

    for hp in range(H // 2):            # head pairs (2hp, 2hp+1)
        for qc in range(QC):
            pc = [ps_ctx.tile([HD + 1, NCH], F32, name=f"pc{j}", tag=f"pc{j}") for j in range(2)]
            for kt in range(ST):
                if hp == 0 and qc == 0:
                    if kt == 0:
                        pe_touch(wo_a[:, 0, :], ps_v)
                    v_proj(kt, ps_v)   # overlap V projection with attention
                # S_T for both heads of the pair into one [128, 2*NCH] psum
                # (head j in columns j*NCH..), so ONE exp covers the pair.
                sps = ps_smm.tile([P, 2 * NCH], F32, name="smm", tag="smm")
                for j in range(2):
                    sl = slice(j * HD, (j + 1) * HD)
                    nc.tensor.matmul(
                        sps[:, j * NCH:(j + 1) * NCH],
                        K_sb[hp][sl, kt * P:(kt + 1) * P],
                        Q_sb[hp][sl, qc * NCH:(qc + 1) * NCH],
                        start=True, stop=True,
                    )
                a_sb = work.tile([P, 2 * NCH], BF16, name="A", tag="A")
                nc.scalar.activation(
                    a_sb[:], sps[:], AF.Exp,
                    bias=mask_sb[:, kt:kt + 1], scale=1.0,
                )
                for j in range(2):
                    h = 2 * hp + j
                    # ctx (+ denominator in row HD): accumulate over k tiles
                    nc.tensor.matmul(
                        pc[j][:],
                        V_sb[kt][:, h * (HD + 1):(h + 1) * (HD + 1)],
                        a_sb[:, j * NCH:(j + 1) * NCH],
                        start=(kt == 0), stop=(kt == ST - 1),
                    )
            # drain psum fast (normalization deferred): raw ctx + denominators
            for j in range(2):
                blk = (hp * QC + qc) * 2 + j
                nc.vector.tensor_copy(
                    craw[hp * QC + qc][j * HD:(j + 1) * HD, :], pc[j][0:HD, :])
                nc.vector.tensor_copy(sums_slot(blk), pc[j][HD:HD + 1, :])

    es_att.close()
    es_xw.close()
    es_kqv.close()

    # softmax denominators: one batched inv = exp(-ln(sum)) for all heads
    nc.scalar.activation(sums_g[:], sums_g[:], AF.Ln)
    nc.scalar.activation(sums_g[:], sums_g[:], AF.Exp, scale=-1.0)
    es_bc2 = ExitStack()
    ps_bc2 = es_bc2.enter_context(
        tc.tile_pool(name="ps_bc2", bufs=2, space="PSUM"))
    for hp in range(H // 2):
        for qc in range(QC):
            for j in range(2):
                blk = (hp * QC + qc) * 2 + j
                bcp = bcast_row(sums_slot(blk), HD, ps_bc2, "bc2")
                nc.vector.tensor_tensor(
                    C_sb[hp][j * HD:(j + 1) * HD, qc * NCH:(qc + 1) * NCH],
                    craw[hp * QC + qc][j * HD:(j + 1) * HD, :], bcp[:], ALU.mult,
                )
    es_bc2.close()
    es_cr.close()

    # ---------------- phase 3: Wo projection + residual + LN1 ----------------
    es_pre = ExitStack()
    prep = es_pre.enter_context(tc.tile_pool(name="prep", bufs=1))
    n1pre = [prep.tile([P, Q], F32, name=f"n1pre{i}", tag=f"pre{i}")
             for i in range(DT)]

    es_wo = ExitStack()
    ps_wo = es_wo.enter_context(
        tc.tile_pool(name="ps_wo", bufs=2, space="PSUM"))
    for ot in range(DT):
        for qc in range(QC):
            ps = ps_wo.tile([P, NCH], F32, name="mm", tag="mm")
            for ct in range(DT):
                nc.tensor.matmul(
                    ps[:],
                    wo_sb[ct][:, ot * P:(ot + 1) * P],
                    C_sb[ct][:, qc * NCH:(qc + 1) * NCH],
                    start=(ct == 0), stop=(ct == DT - 1),
                )
            nc.vector.tensor_add(
                n1pre[ot][:, qc * NCH:(qc + 1) * NCH], ps[:],
                xq_sb[ot][:, qc * NCH:(qc + 1) * NCH],
            )

    es_wo.close()
    es_c.close()

    es_n1 = ExitStack()
    n1pool = es_n1.enter_context(tc.tile_pool(name="n1", bufs=1, side="right"))
    n1f = [n1pool.tile([P, Q], F32, name=f"n1f{i}", tag=f"n1f{i}") for i in range(DT)]
    n1bf = [n1pool.tile([P, Q], BF16, name=f"n1bf{i}", tag=f"n1bf{i}")
            for i in range(DT)]

    def layer_norm(src_tiles, ps_stats, ps_bcln, get_dst, emit_out):
        """Feature-major LayerNorm over the partition (feature) axis.

        Stats via ones-matmuls; rstd via exp(-0.5 ln(var+eps)); normalization
        applied with broadcast tiles built by K<=2 matmuls. emit_out(ot, qc,
        normed_ap) consumes the normalized fp32 [P, NCH] slice."""
        for qc in range(QC):
            qsl = slice(qc * NCH, (qc + 1) * NCH)
            mu_ps = ps_stats.tile([1, NCH], F32, name="smu", tag="smu")
            sq_ps = ps_stats.tile([1, NCH], F32, name="ssq", tag="ssq")
            for ot in range(DT):
                nb = work.tile([P, NCH], BF16, name="nb", tag="nb", bufs=3)
                nc.vector.tensor_copy(nb[:], src_tiles[ot][:, qsl])
                nc.tensor.matmul(
                    mu_ps[:], ones_bf[:], nb[:],
                    start=(ot == 0), stop=(ot == DT - 1),
                )
                sq = work.tile([P, NCH], BF16, name="sq", tag="sq", bufs=3)
                nc.scalar.activation(sq[:], src_tiles[ot][:, qsl], AF.Square)
                nc.tensor.matmul(
                    sq_ps[:], ones_bf[:], sq[:],
                    start=(ot == 0), stop=(ot == DT - 1),
                )
            mu = stat.tile([1, NCH], F32, name="mu", tag="mu", bufs=1)
            nc.vector.tensor_scalar_mul(mu[:], mu_ps[:], 1.0 / D)
            t2 = stat.tile([1, NCH], F32, name="t2", tag="t2", bufs=1)
            nc.vector.tensor_scalar_mul(t2[:], sq_ps[:], 1.0 / D)
            musq = stat.tile([1, NCH], F32, name="musq", tag="musq", bufs=1)
            nc.vector.tensor_tensor(musq[:], mu[:], mu[:], ALU.mult)
            nc.vector.tensor_tensor(t2[:], t2[:], musq[:], ALU.subtract)
            # rstd = exp(-0.5 * ln(var + eps)), computed in place
            nc.scalar.activation(t2[:], t2[:], AF.Ln, bias=eps1[:])
            nc.scalar.activation(t2[:], t2[:], AF.Exp, scale=-0.5)
            rstd = t2
            # broadcast mu and rstd to [P, NCH] (exact hi/lo bf16)
            bmu = bcast_row(mu[:], P, ps_bcln, "bmu")
            brstd = bcast_row(rstd[:], P, ps_bcln, "brstd")
            for ot in range(DT):
                dst = get_dst(ot, qc)
                nc.vector.tensor_tensor(dst, src_tiles[ot][:, qsl], bmu[:], ALU.subtract)
                nc.vector.tensor_tensor(dst, dst, brstd[:], ALU.mult)
                nc.vector.tensor_scalar(
                    dst, dst,
                    lng_sb[:, ot:ot + 1], lnb_sb[:, ot:ot + 1], ALU.mult, ALU.add,
                )
                emit_out(ot, qc, dst)

    es_ln = ExitStack()
    ps_stats = es_ln.enter_context(tc.tile_pool(name="ps_st", bufs=1, space="PSUM", side="right"))
    ps_bcln = es_ln.enter_context(tc.tile_pool(name="ps_bl", bufs=1, space="PSUM", side="right"))

    def ln1_dst(ot, qc):
        return n1f[ot][:, qc * NCH:(qc + 1) * NCH]

    def ln1_out(ot, qc, dst):
        nc.vector.tensor_copy(n1bf[ot][:, qc * NCH:(qc + 1) * NCH], dst)

    layer_norm(n1pre, ps_stats, ps_bcln, ln1_dst, ln1_out)

    # ---------------- phase 4: FFN ----------------
    es_h = ExitStack()
    ps_h = es_h.enter_context(tc.tile_pool(name="ps_h", bufs=2, space="PSUM"))
    hpool = es_h.enter_context(tc.tile_pool(name="hpool", bufs=1))
    h_sb = [hpool.tile([P, Q], BF16, name=f"h{i}", tag=f"h{i}") for i in range(FT)]
    es_w1 = ExitStack()
    w1s = es_w1.enter_context(tc.tile_pool(name="w1s", bufs=2))

    W1CH = 256
    for fc in range(F // W1CH):           # 12 chunks of 256 ffn features
        w1c = w1s.tile([P, DT, W1CH], BF16, name="w1c", tag="w1c")
        nc.sync.dma_start(
            out=w1c[:],
            in_=w1T_r[:, :, fc * W1CH:(fc + 1) * W1CH].rearrange("t p m -> p t m"),
        )
        if fc == 0:
            # tiny PE touch: absorbs the SBUF-zone-release + DMA waits into
            # the PE clock so the real matmuls stay within the 2-wait limit
            tps = ps_h.tile([1, 8], F32, name="tch1", tag="hmm")
            nc.tensor.matmul(tps[:], w1c[0:1, 0, 0:1], w1c[0:1, 0, 0:8],
                             start=True, stop=True)
        for fl in range(W1CH // P):       # 2 ffn tiles per chunk
            ft = fc * 2 + fl
            for qc in range(QC):
                ps = ps_h.tile([P, NCH], F32, name="hmm", tag="hmm")
                for kt in range(DT):
                    nc.tensor.matmul(
                        ps[:],
                        w1c[:, kt, fl * P:(fl + 1) * P],
                        n1bf[kt][:, qc * NCH:(qc + 1) * NCH],
                        start=(kt == 0), stop=(kt == DT - 1),
                    )
                nc.scalar.activation(
                    h_sb[ft][:, qc * NCH:(qc + 1) * NCH], ps[:], AF.Gelu,
                )

    es_w1.close()

    es_w2 = ExitStack()
    ps_f = es_w2.enter_context(tc.tile_pool(name="ps_f", bufs=2, space="PSUM"))
    w2s = es_w2.enter_context(tc.tile_pool(name="w2s", bufs=2, side="right"))
    n2pre = [prep.tile([P, Q], F32, name=f"n2pre{i}", tag=f"pre{i}")
             for i in range(DT)]

    W2CH = 256
    for oc in range(D // W2CH):           # 3 chunks of 256 output features
        w2c = w2s.tile([P, FT, W2CH], BF16, name="w2c", tag="w2c")
        nc.sync.dma_start(
            out=w2c[:],
            in_=w2T_r[:, :, oc * W2CH:(oc + 1) * W2CH].rearrange("t p m -> p t m"),
        )
        if oc == 0:
            tps = ps_f.tile([1, 8], F32, name="tch2", tag="fmm")
            nc.tensor.matmul(tps[:], w2c[0:1, 0, 0:1], w2c[0:1, 0, 0:8],
                             start=True, stop=True)
        for ol in range(W2CH // P):
            ot = oc * 2 + ol
            for qc in range(QC):
                ps = ps_f.tile([P, NCH], F32, name="fmm", tag="fmm")
                for ft in range(FT):
                    nc.tensor.matmul(
                        ps[:],
                        w2c[:, ft, ol * P:(ol + 1) * P],
                        h_sb[ft][:, qc * NCH:(qc + 1) * NCH],
                        start=(ft == 0), stop=(ft == FT - 1),
                    )
                nc.vector.tensor_add(
                    n2pre[ot][:, qc * NCH:(qc + 1) * NCH], ps[:],
                    n1f[ot][:, qc * NCH:(qc + 1) * NCH],
                )

    es_w2.close()
    es_h.close()
    es_n1.close()

    # ---------------- phase 5: LN2 + store ----------------
    _otiles = {}

    def ln2_dst(ot, qc):
        o = outp.tile([P, NCH], F32, name="o", tag="o", bufs=2)
        _otiles[(ot, qc)] = o
        return o[:]

    def ln2_out(ot, qc, dst):
        nc.sync.dma_start(out=outT_r[ot][:, qc * NCH:(qc + 1) * NCH],
                          in_=_otiles[(ot, qc)][:])

    layer_norm(n2pre, ps_stats, ps_bcln, ln2_dst, ln2_out)

    es_ln.close()
    es_pre.close()
    es.close()


class _EncBacc(bacc.Bacc):
    """Bacc whose activation-table pass never picks the exp-only set, so Exp
    and Ln co-reside in natural_log_exp_and_others (no table thrashing)."""

    def insert_act_table_loads(self):
        import bass_rust as _bass_rust
        from concourse.hw_specs import get_activation_tables
        has_activation = any(
            isinstance(i, mybir.InstActivation)
            for b in self.main_func.blocks
            for i in b.instructions
        )
        if not has_activation:
            return
        AFT = mybir.ActivationFunctionType
        tables = []
        for name, funcs in get_activation_tables(self.m.arch).items():
            if name in ("exp_and_others", "exp_and_friends"):
                funcs = funcs - {AFT.Exp}
            tables.append((name, funcs))
        _bass_rust.insert_act_table_loads(self, tables)


def build():
    nc = _EncBacc("TRN2", target_bir_lowering=False, debug=False)
    t = {}
    t["xT"] = nc.dram_tensor("xT", [D, S], F32, kind="ExternalInput").ap()
    for name, shape in [
        ("wqT", [D, D]), ("wkT", [D, D]), ("wvT", [D, D]), ("woT", [D, D]),
        ("w1T", [D, F]), ("w2T", [F, D]),
    ]:
        t[name] = nc.dram_tensor(name, shape, BF16, kind="ExternalInput").ap()
    t["maskc"] = nc.dram_tensor("maskc", [P, ST], F32, kind="ExternalInput").ap()
    t["lng"] = nc.dram_tensor("lng", [P, DT], F32, kind="ExternalInput").ap()
    t["lnb"] = nc.dram_tensor("lnb", [P, DT], F32, kind="ExternalInput").ap()
    t["outT"] = nc.dram_tensor("outT", [D, Q], F32, kind="ExternalOutput").ap()

    with tile.TileContext(nc) as tc:
        _emit(nc, tc, t)
    nc.compile()
    return nc


_NC_CACHE = None


def _get_nc():
    global _NC_CACHE
    if _NC_CACHE is None:
        _NC_CACHE = build()
    return _NC_CACHE


def make_in_maps(hidden_state, key_padding_mask, Wq, Wk, Wv, Wo, W1, W2, ln_g, ln_b):
    x = np.asarray(hidden_state, dtype=np.float32)
    mask = np.asarray(key_padding_mask)
    bf = ml_dtypes.bfloat16

    shared = {
        "wqT": (np.asarray(Wq, np.float32).T * 0.125).astype(bf),
        "wkT": np.asarray(Wk, np.float32).T.astype(bf),
        "wvT": np.asarray(Wv, np.float32).T.astype(bf),
        "woT": np.asarray(Wo, np.float32).T.astype(bf),
        "w1T": np.asarray(W1, np.float32).T.astype(bf),
        "w2T": np.asarray(W2, np.float32).T.astype(bf),
        "lng": np.ascontiguousarray(
            np.asarray(ln_g, np.float32).reshape(DT, P).T),
        "lnb": np.ascontiguousarray(
            np.asarray(ln_b, np.float32).reshape(DT, P).T),
    }
    shared = {k: np.ascontiguousarray(v) for k, v in shared.items()}

    maskadd = np.where(mask == 0, np.float32(MASK_NEG), np.float32(0.0))

    in_maps = []
    for c in range(N_CORES):
        b, half = c // 2, c % 2
        perm = np.r_[np.arange(half * Q, (half + 1) * Q),
                     np.arange((1 - half) * Q, (2 - half) * Q)]
        m = dict(shared)
        m["xT"] = np.ascontiguousarray(x[b].T[:, perm])
        m["maskc"] = np.ascontiguousarray(
            maskadd[b][perm].astype(np.float32).reshape(ST, P).T)
        in_maps.append(m)
    return in_maps


def gather(results, N=4):
    out = np.empty((N, S, D), dtype=np.float32)
    for c in range(N_CORES):
        b, half = c // 2, c % 2
        out[b, half * Q:(half + 1) * Q, :] = results[c]["outT"].T
    return out


def _install_ntff_hook():
    """Provide antenv.axon_hooks (absent in this image) so that
    run_bass_kernel_spmd(trace=True) can profile via the axon .so."""
    import types
    import antenv
    if getattr(antenv, "axon_hooks", None) is not None:
        return
    from trn_agent_boot.trn_boot import _ntff_profile_via_ctypes
    hook = _ntff_profile_via_ctypes("/opt/axon/libaxon_pjrt.so")
    mod = types.ModuleType("antenv.axon_hooks")
    mod._hook = hook
    mod.get_axon_ntff_profile_hook = lambda: mod._hook
    mod.set_axon_ntff_profile_hook = lambda h: setattr(mod, "_hook", h)
    sys.modules["antenv.axon_hooks"] = mod
    antenv.axon_hooks = mod


def kernel(hidden_state, key_padding_mask, Wq, Wk, Wv, Wo, W1, W2, ln_g, ln_b,
           trace=False):
    from concourse.bass_utils import run_bass_kernel_spmd

    if trace:
        _install_ntff_hook()

    nc = _get_nc()
    in_maps = make_in_maps(hidden_state, key_padding_mask,
                           Wq, Wk, Wv, Wo, W1, W2, ln_g, ln_b)
    res = run_bass_kernel_spmd(nc, in_maps, core_ids=list(range(N_CORES)),
                               trace=trace)
    out = gather(res.results)
    if trace:
        return out, res
    return out


# revision 33
# speedup vs baseline: 1.1039x; 1.1039x over previous
"""Trainium2 Bass kernel for nn_Encoder_75634374083175.

Single transformer encoder layer: MHA (12 heads, D=768) + LayerNorm + FFN(3072,
exact GELU) + LayerNorm, on [4, 2048, 768] fp32 input with an int key-padding
mask.

Sharding: 8-way data parallel over (batch, half-sequence). Core c handles batch
c//2 and query rows [half*1024, (half+1)*1024). Each core re-projects K/V for
its full batch (duplicated across the core pair) so there are no collectives.

Device layout: all activations are kept "feature-major" ([feature, token] with
features on SBUF partitions) so every matmul uses the weight as the stationary
operand and the activation as the moving operand. Scores are computed directly
transposed (S_T[k_tok, q_tok]), so the softmax exp's additive mask is a
per-partition ACT bias, and the unnormalized attention matrix feeds the context
matmul with no transposes anywhere. Softmax denominators come from a ones
column appended to V (fused into the context matmul); reciprocals and rsqrt
are computed as exp(-ln(x)) on the ACT engine (single table set).
"""

import sys

sys.path.insert(0, "/opt/trn_rl_repo")

import numpy as np
import ml_dtypes

import concourse.bass as bass
import concourse.tile as tile
from concourse import bacc
from concourse import mybir
from contextlib import ExitStack

F32 = mybir.dt.float32
F32R = mybir.dt.float32r
BF16 = mybir.dt.bfloat16
AF = mybir.ActivationFunctionType
ALU = mybir.AluOpType

D = 768          # hidden
H = 12           # heads
HD = 64          # head dim
F = 3072         # ffn hidden
S = 2048         # full sequence per batch
Q = 1024         # query rows per core
P = 128          # partitions
DT = D // P      # 6 feature tiles
ST = S // P      # 16 key-token tiles
FT = F // P      # 24 ffn feature tiles
NCH = 512        # matmul moving free-dim chunk
QC = Q // NCH    # 2 query chunks
KC = S // NCH    # 4 key-token chunks
LN_EPS = 1e-5
MASK_NEG = -30000.0

N_CORES = 8


def _emit(nc, tc, t):
    """Emit the whole encoder layer under TileContext `tc`.

    `t` is a dict of DRAM APs (see build())."""
    es = ExitStack()

    xT_r = t["xT"].rearrange("(t p) m -> t p m", p=P)        # [6,128,2048] f32
    wqT_r = t["wqT"].rearrange("(t p) m -> t p m", p=P)      # [6,128,768] bf16
    wkT_r = t["wkT"].rearrange("(t p) m -> t p m", p=P)
    wvT_r = t["wvT"].rearrange("(t p) m -> t p m", p=P)
    woT_r = t["woT"].rearrange("(t p) m -> t p m", p=P)
    w1T_r = t["w1T"].rearrange("(t p) m -> t p m", p=P)      # [6,128,3072]
    w2T_r = t["w2T"].rearrange("(t p) m -> t p m", p=P)      # [24,128,768]
    outT_r = t["outT"].rearrange("(t p) m -> t p m", p=P)    # [6,128,1024] f32

    # ---------------- long-lived pools ----------------
    consts = es.enter_context(tc.tile_pool(name="consts", bufs=1))
    work = es.enter_context(tc.tile_pool(name="work", bufs=4))
    stat = es.enter_context(tc.tile_pool(name="stat", bufs=2))
    outp = es.enter_context(tc.tile_pool(name="outp", bufs=4))

    mask_sb = consts.tile([P, ST], F32, name="mask", tag="mask")
    nc.sync.dma_start(out=mask_sb[:], in_=t["maskc"][:])
    lng_sb = consts.tile([P, DT], F32, name="lng", tag="lng")
    nc.sync.dma_start(out=lng_sb[:], in_=t["lng"][:])
    lnb_sb = consts.tile([P, DT], F32, name="lnb", tag="lnb")
    nc.sync.dma_start(out=lnb_sb[:], in_=t["lnb"][:])
    ones_f = consts.tile([P, P], F32, name="ones_f", tag="ones_f")
    nc.vector.memset(ones_f[:], 1.0)
    ones_bf = consts.tile([P, 1], BF16, name="ones_bf", tag="ones_bf")
    nc.vector.memset(ones_bf[:], 1.0)
    ones_bfr = consts.tile([1, P], BF16, name="ones_bfr", tag="ones_bfr")
    nc.vector.memset(ones_bfr[:], 1.0)
    ones_bq = consts.tile([P, P], BF16, name="ones_bq", tag="ones_bq")
    nc.vector.memset(ones_bq[:], 1.0)
    eps1 = consts.tile([1, 1], F32, name="eps1", tag="eps1")
    nc.vector.memset(eps1[:], LN_EPS)

    def pe_touch(ap_row, pool, tag="mm"):
        """Tiny matmul reading ap_row ([1,>=8] slice) so PE observes its
        producer's semaphore; keeps DMA waits off real matmuls."""
        tps = pool.tile([1, 8], F32, name="tch", tag=tag)
        nc.tensor.matmul(tps[:], ap_row[0:1, 0:1], ap_row[0:1, 0:8],
                         start=True, stop=True)

    def bcast_row(src_row, rows, ps_pool, tag):
        """[1,N] f32 -> psum [rows,N] f32 broadcast, exact via hi+lo bf16."""
        srow = work.tile([1, NCH], F32, name="brow", tag="brow", bufs=2)
        nc.vector.tensor_copy(srow[:], src_row)
        hi = work.tile([1, NCH], BF16, name="bchi", tag="bchi", bufs=2)
        nc.vector.tensor_copy(hi[:], srow[:])
        lo32 = work.tile([1, NCH], F32, name="bclo32", tag="bclo32", bufs=2)
        nc.vector.tensor_tensor(lo32[:], srow[:], hi[:], ALU.subtract)
        lo = work.tile([1, NCH], BF16, name="bclo", tag="bclo", bufs=2)
        nc.vector.tensor_copy(lo[:], lo32[:])
        bc = ps_pool.tile([rows, NCH], F32, name=tag, tag=tag)
        nc.tensor.matmul(bc[:], ones_bfr[0:1, 0:rows], hi[:],
                         start=True, stop=False)
        nc.tensor.matmul(bc[:], ones_bfr[0:1, 0:rows], lo[:],
                         start=False, stop=True)
        return bc

    # ---------------- phase 1: projections ----------------
    es_kqv = ExitStack()
    kqv = es_kqv.enter_context(tc.tile_pool(name="kqv", bufs=1))
    K_sb = [kqv.tile([P, S], BF16, name=f"K{i}", tag=f"K{i}") for i in range(DT)]
    Q_sb = [kqv.tile([P, Q], BF16, name=f"Q{i}", tag=f"Q{i}") for i in range(DT)]
    # V token-major with a ones column per head: [k_tok, 12*(64+1)]
    V_sb = [kqv.tile([P, H * (HD + 1)], BF16, name=f"V{i}", tag=f"V{i}") for i in range(ST)]

    es_xw = ExitStack()
    es_pp = ExitStack()
    ps_proj = es_pp.enter_context(
        tc.tile_pool(name="ps_proj", bufs=2, space="PSUM"))
    xw = es_xw.enter_context(tc.tile_pool(name="xw", bufs=1))
    wqk = es_xw.enter_context(tc.tile_pool(name="wqk", bufs=1))
    # single tiles + single DMAs so consumers wait on few semaphores
    xbf_a = xw.tile([P, DT, S], BF16, name="xbf", tag="xbf")
    wq_a = wqk.tile([P, DT, D], BF16, name="wqa", tag="wqa")
    wk_a = wqk.tile([P, DT, D], BF16, name="wka", tag="wka")
    wv_a = xw.tile([P, DT, D], BF16, name="wva", tag="wva")
    # f32 -> bf16 cast during DMA (SWDGE)
    nc.gpsimd.dma_start(out=xbf_a[:], in_=t["xT"].rearrange("(t p) m -> p t m", p=P))
    nc.sync.dma_start(out=wk_a[:], in_=t["wkT"].rearrange("(t p) m -> p t m", p=P))
    nc.sync.dma_start(out=wv_a[:], in_=t["wvT"].rearrange("(t p) m -> p t m", p=P))
    nc.sync.dma_start(out=wq_a[:], in_=t["wqT"].rearrange("(t p) m -> p t m", p=P))
    pe_touch(xbf_a[:, 0, :], ps_proj)
    pe_touch(wk_a[:, 0, :], ps_proj)
    pe_touch(wv_a[:, 0, :], ps_proj)
    pe_touch(wq_a[:, 0, :], ps_proj)
    xbf = [xbf_a[:, i, :] for i in range(DT)]
    wq_sb = [wq_a[:, i, :] for i in range(DT)]
    wk_sb = [wk_a[:, i, :] for i in range(DT)]
    wv_sb = [wv_a[:, i, :] for i in range(DT)]

    # K_T[f, k_tok] = sum_d WkT[d, f] * xT[d, k_tok]
    def k_proj(mt, ck, pool):
        ps = pool.tile([P, NCH], F32, name="mm", tag="mm")
        for kt in range(DT):
            nc.tensor.matmul(
                ps[:],
                wk_sb[kt][:, mt * P:(mt + 1) * P],
                xbf[kt][:, ck * NCH:(ck + 1) * NCH],
                start=(kt == 0), stop=(kt == DT - 1),
            )
        nc.vector.tensor_copy(K_sb[mt][:, ck * NCH:(ck + 1) * NCH], ps[:])

    def v_proj(tt, pool):
        # V token-major: V[k_tok, f] = sum_d xT[d, k_tok] * WvT[d, f]
        for (c0, c1) in ((0, NCH), (NCH, D)):
            w = c1 - c0
            ps = pool.tile([P, NCH], F32, name="mm", tag="mm")
            for kt in range(DT):
                nc.tensor.matmul(
                    ps[:, 0:w],
                    xbf[kt][:, tt * P:(tt + 1) * P],
                    wv_sb[kt][:, c0:c1],
                    start=(kt == 0), stop=(kt == DT - 1),
                )
            # scatter per head: psum cols are v-features (c0..c1)
            h0 = c0 // HD
            nh = w // HD
            dst = V_sb[tt][:, h0 * (HD + 1):(h0 + nh) * (HD + 1)]
            dst = dst.rearrange("p (h c) -> p h c", c=HD + 1)[:, :, 0:HD]
            src = ps[:, 0:w].rearrange("p (h c) -> p h c", c=HD)
            nc.vector.tensor_copy(dst, src)
        # ones column per head (softmax denominator accumulator)
        oc = V_sb[tt].rearrange("p (h c) -> p h c", c=HD + 1)[:, :, HD:HD + 1]
        nc.vector.memset(oc, 1.0)

    # Q_T[f, q] (queries are the first Q tokens; wqT pre-scaled by 1/8)
    def q_proj(mt, qcc, pool):
        ps = pool.tile([P, NCH], F32, name="mm", tag="mm")
        for kt in range(DT):
            nc.tensor.matmul(
                ps[:],
                wq_sb[kt][:, mt * P:(mt + 1) * P],
                xbf[kt][:, qcc * NCH:(qcc + 1) * NCH],
                start=(kt == 0), stop=(kt == DT - 1),
            )
        nc.vector.tensor_copy(Q_sb[mt][:, qcc * NCH:(qcc + 1) * NCH], ps[:])

    # only head-pairs 0,1 projected up front; the rest interleave with the
    # attention loop to keep PE fed while ACT grinds exps
    for mt in (0, 1):
        for ck in range(KC):
            k_proj(mt, ck, ps_proj)
        for qcc in range(QC):
            q_proj(mt, qcc, ps_proj)

    es_pp.close()

    # ---------------- phase 2: attention ----------------
    es_c = ExitStack()
    cpool = es_c.enter_context(tc.tile_pool(name="cpool", bufs=1, side="right"))
    C_sb = [cpool.tile([P, Q], BF16, name=f"C{i}", tag=f"C{i}") for i in range(DT)]
    wo_a = cpool.tile([P, DT, D], BF16, name="woa", tag="woa")
    nc.sync.dma_start(out=wo_a[:], in_=t["woT"].rearrange("(t p) m -> p t m", p=P))
    wo_sb = [wo_a[:, i, :] for i in range(DT)]

    es_att = ExitStack()
    ps_v = es_att.enter_context(
        tc.tile_pool(name="ps_v", bufs=1, space="PSUM"))
    ps_smm = es_att.enter_context(
        tc.tile_pool(name="ps_smm", bufs=2, space="PSUM"))
    ps_ctx = es_att.enter_context(tc.tile_pool(name="ps_ctx", bufs=1, space="PSUM", side="right"))

    es_cr = ExitStack()
    crp = es_cr.enter_context(tc.tile_pool(name="crp", bufs=1, side="right"))
    craw = [crp.tile([P, NCH], BF16, name=f"craw{i}", tag=f"craw{i}")
            for i in range(H)]                  # (hp, qc) pair tiles
    # 24 denominator rows in a [128, 8*NCH] grid: rows {0,32,64} x 8 cols
    # (row 96 avoided: PE quadrant-3 streaming is broken in hardware)
    sums_g = crp.tile([P, 8 * NCH], F32, name="sums", tag="sums")
    nc.vector.memset(sums_g[:], 1.0)

    def sums_slot(blk):
        r = 32 * (blk % 3)
        c = (blk // 3) * NCH
        return sums_g[r:r + 1, c:c + NCH]

    for hp in range(H // 2):            # head pairs (2hp, 2hp+1)
        for qc in range(QC):
            pc = [ps_ctx.tile([HD + 1, NCH], F32, name=f"pc{j}", tag=f"pc{j}") for j in range(2)]
            for kt in range(ST):
                if hp == 0 and qc == 0:
                    if kt == 0:
                        pe_touch(wo_a[:, 0, :], ps_v)
                    v_proj(kt, ps_v)   # overlap V projection with attention
                if 1 <= hp <= 4 and qc == 0:
                    mt = hp + 1       # project K/Q for a later head pair
                    if kt in (0, 3, 6, 9):
                        k_proj(mt, kt // 3, ps_v)
                    elif kt in (12, 14):
                        q_proj(mt, (kt - 12) // 2, ps_v)
                # S_T for both heads of the pair into one [128, 2*NCH] psum
                # (head j in columns j*NCH..), so ONE exp covers the pair.
                sps = ps_smm.tile([P, 2 * NCH], F32, name="smm", tag="smm")
                for j in range(2):
                    sl = slice(j * HD, (j + 1) * HD)
                    nc.tensor.matmul(
                        sps[:, j * NCH:(j + 1) * NCH],
                        K_sb[hp][sl, kt * P:(kt + 1) * P],
                        Q_sb[hp][sl, qc * NCH:(qc + 1) * NCH],
                        start=True, stop=True,
                    )
                a_sb = work.tile([P, 2 * NCH], BF16, name="A", tag="A")
                nc.scalar.activation(
                    a_sb[:], sps[:], AF.Exp,
                    bias=mask_sb[:, kt:kt + 1], scale=1.0,
                )
                for j in range(2):
                    h = 2 * hp + j
                    # ctx (+ denominator in row HD): accumulate over k tiles
                    nc.tensor.matmul(
                        pc[j][:],
                        V_sb[kt][:, h * (HD + 1):(h + 1) * (HD + 1)],
                        a_sb[:, j * NCH:(j + 1) * NCH],
                        start=(kt == 0), stop=(kt == ST - 1),
                    )
            # drain psum fast (normalization deferred): raw ctx + denominators
            for j in range(2):
                blk = (hp * QC + qc) * 2 + j
                nc.vector.tensor_copy(
                    craw[hp * QC + qc][j * HD:(j + 1) * HD, :], pc[j][0:HD, :])
                nc.vector.tensor_copy(sums_slot(blk), pc[j][HD:HD + 1, :])

    es_att.close()
    es_xw.close()
    es_kqv.close()

    es_pre = ExitStack()
    prep = es_pre.enter_context(tc.tile_pool(name="prep", bufs=1))
    n1pre = [prep.tile([P, Q], F32, name=f"n1pre{i}", tag=f"pre{i}")
             for i in range(DT)]

    es_xq = ExitStack()
    xqp = es_xq.enter_context(tc.tile_pool(name="xqp", bufs=1))
    xq_a = xqp.tile([P, DT, Q], F32, name="xqa", tag="xqa")
    nc.sync.dma_start(
        out=xq_a[:],
        in_=t["xT"].rearrange("(t p) m -> p t m", p=P)[:, :, 0:Q])
    xq_sb = [xq_a[:, i, :] for i in range(DT)]

    # softmax denominators: one batched inv = exp(-ln(sum)) for all heads,
    # then one batched hi/lo bf16 split of the whole grid
    nc.scalar.activation(sums_g[:], sums_g[:], AF.Ln)
    nc.scalar.activation(sums_g[:], sums_g[:], AF.Exp, scale=-1.0)
    es_bc2 = ExitStack()
    hilo = es_bc2.enter_context(tc.tile_pool(name="hilo", bufs=1))
    hi_g = hilo.tile([P, 8 * NCH], BF16, name="hig", tag="hig")
    nc.vector.tensor_copy(hi_g[:], sums_g[:])
    lo32_g = hilo.tile([P, 8 * NCH], F32, name="lo32g", tag="lo32g")
    nc.vector.tensor_tensor(lo32_g[:], sums_g[:], hi_g[:], ALU.subtract)
    lo_g = hilo.tile([P, 8 * NCH], BF16, name="log", tag="log")
    nc.vector.tensor_copy(lo_g[:], lo32_g[:])
    ps_bc2 = es_bc2.enter_context(
        tc.tile_pool(name="ps_bc2", bufs=4, space="PSUM"))
    for hp in range(H // 2):
        for qc in range(QC):
            for j in range(2):
                blk = (hp * QC + qc) * 2 + j
                r = 32 * (blk % 3)
                c = (blk // 3) * NCH
                bcp = ps_bc2.tile([HD, NCH], F32, name="bc2", tag="bc2")
                nc.tensor.matmul(bcp[:], ones_bq[r:r + 1, 0:HD],
                                 hi_g[r:r + 1, c:c + NCH],
                                 start=True, stop=False, tile_position=(r, 0))
                nc.tensor.matmul(bcp[:], ones_bq[r:r + 1, 0:HD],
                                 lo_g[r:r + 1, c:c + NCH],
                                 start=False, stop=True, tile_position=(r, 0))
                nc.vector.tensor_tensor(
                    C_sb[hp][j * HD:(j + 1) * HD, qc * NCH:(qc + 1) * NCH],
                    craw[hp * QC + qc][j * HD:(j + 1) * HD, :], bcp[:], ALU.mult,
                )
    es_bc2.close()
    es_cr.close()

    # ---------------- phase 3: Wo projection + residual + LN1 ----------------
    es_wo = ExitStack()
    ps_wo = es_wo.enter_context(
        tc.tile_pool(name="ps_wo", bufs=2, space="PSUM"))
    for ot in range(DT):
        for qc in range(QC):
            ps = ps_wo.tile([P, NCH], F32, name="mm", tag="mm")
            for ct in range(DT):
                nc.tensor.matmul(
                    ps[:],
                    wo_sb[ct][:, ot * P:(ot + 1) * P],
                    C_sb[ct][:, qc * NCH:(qc + 1) * NCH],
                    start=(ct == 0), stop=(ct == DT - 1),
                )
            nc.vector.tensor_add(
                n1pre[ot][:, qc * NCH:(qc + 1) * NCH], ps[:],
                xq_sb[ot][:, qc * NCH:(qc + 1) * NCH],
            )

    es_wo.close()
    es_xq.close()
    es_c.close()

    es_n1 = ExitStack()
    n1pool = es_n1.enter_context(tc.tile_pool(name="n1", bufs=1, side="right"))
    n1f = [n1pool.tile([P, Q], F32, name=f"n1f{i}", tag=f"n1f{i}") for i in range(DT)]
    n1bf = [n1pool.tile([P, Q], BF16, name=f"n1bf{i}", tag=f"n1bf{i}")
            for i in range(DT)]

    def layer_norm(src_tiles, ps_stats, ps_bcln, get_dst, emit_out):
        """Feature-major LayerNorm over the partition (feature) axis.

        Stats via ones-matmuls; rstd via exp(-0.5 ln(var+eps)); normalization
        applied with broadcast tiles built by K<=2 matmuls. emit_out(ot, qc,
        normed_ap) consumes the normalized fp32 [P, NCH] slice."""
        for qc in range(QC):
            qsl = slice(qc * NCH, (qc + 1) * NCH)
            mu_ps = ps_stats.tile([1, NCH], F32, name="smu", tag="smu")
            sq_ps = ps_stats.tile([1, NCH], F32, name="ssq", tag="ssq")
            for ot in range(DT):
                nb = work.tile([P, NCH], BF16, name="nb", tag="nb", bufs=3)
                nc.vector.tensor_copy(nb[:], src_tiles[ot][:, qsl])
                nc.tensor.matmul(
                    mu_ps[:], ones_bf[:], nb[:],
                    start=(ot == 0), stop=(ot == DT - 1),
                )
                sq = work.tile([P, NCH], BF16, name="sq", tag="sq", bufs=3)
                nc.scalar.activation(sq[:], src_tiles[ot][:, qsl], AF.Square)
                nc.tensor.matmul(
                    sq_ps[:], ones_bf[:], sq[:],
                    start=(ot == 0), stop=(ot == DT - 1),
                )
            mu = stat.tile([1, NCH], F32, name="mu", tag="mu", bufs=1)
            nc.vector.tensor_scalar_mul(mu[:], mu_ps[:], 1.0 / D)
            t2 = stat.tile([1, NCH], F32, name="t2", tag="t2", bufs=1)
            nc.vector.tensor_scalar_mul(t2[:], sq_ps[:], 1.0 / D)
            musq = stat.tile([1, NCH], F32, name="musq", tag="musq", bufs=1)
            nc.vector.tensor_tensor(musq[:], mu[:], mu[:], ALU.mult)
            nc.vector.tensor_tensor(t2[:], t2[:], musq[:], ALU.subtract)
            # rstd = exp(-0.5 * ln(var + eps)), computed in place
            nc.scalar.activation(t2[:], t2[:], AF.Ln, bias=eps1[:])
            nc.scalar.activation(t2[:], t2[:], AF.Exp, scale=-0.5)
            rstd = t2
            # broadcast mu and rstd to [P, NCH] (exact hi/lo bf16)
            bmu = bcast_row(mu[:], P, ps_bcln, "bmu")
            brstd = bcast_row(rstd[:], P, ps_bcln, "brstd")
            for ot in range(DT):
                dst = get_dst(ot, qc)
                nc.vector.tensor_tensor(dst, src_tiles[ot][:, qsl], bmu[:], ALU.subtract)
                nc.vector.tensor_tensor(dst, dst, brstd[:], ALU.mult)
                nc.vector.tensor_scalar(
                    dst, dst,
                    lng_sb[:, ot:ot + 1], lnb_sb[:, ot:ot + 1], ALU.mult, ALU.add,
                )
                emit_out(ot, qc, dst)

    es_ln = ExitStack()
    ps_stats = es_ln.enter_context(tc.tile_pool(name="ps_st", bufs=1, space="PSUM", side="right"))
    ps_bcln = es_ln.enter_context(tc.tile_pool(name="ps_bl", bufs=1, space="PSUM", side="right"))

    def ln1_dst(ot, qc):
        return n1f[ot][:, qc * NCH:(qc + 1) * NCH]

    def ln1_out(ot, qc, dst):
        nc.vector.tensor_copy(n1bf[ot][:, qc * NCH:(qc + 1) * NCH], dst)

    layer_norm(n1pre, ps_stats, ps_bcln, ln1_dst, ln1_out)

    # ---------------- phase 4: FFN ----------------
    es_h = ExitStack()
    ps_h = es_h.enter_context(tc.tile_pool(name="ps_h", bufs=2, space="PSUM"))
    hpool = es_h.enter_context(tc.tile_pool(name="hpool", bufs=1))
    h_sb = [hpool.tile([P, Q], BF16, name=f"h{i}", tag=f"h{i}") for i in range(FT)]
    es_w1 = ExitStack()
    w1s = es_w1.enter_context(tc.tile_pool(name="w1s", bufs=2))

    W1CH = 256
    for fc in range(F // W1CH):           # 12 chunks of 256 ffn features
        w1c = w1s.tile([P, DT, W1CH], BF16, name="w1c", tag="w1c")
        nc.sync.dma_start(
            out=w1c[:],
            in_=w1T_r[:, :, fc * W1CH:(fc + 1) * W1CH].rearrange("t p m -> p t m"),
        )
        if fc == 0:
            # tiny PE touch: absorbs the SBUF-zone-release + DMA waits into
            # the PE clock so the real matmuls stay within the 2-wait limit
            tps = ps_h.tile([1, 8], F32, name="tch1", tag="hmm")
            nc.tensor.matmul(tps[:], w1c[0:1, 0, 0:1], w1c[0:1, 0, 0:8],
                             start=True, stop=True)
        for fl in range(W1CH // P):       # 2 ffn tiles per chunk
            ft = fc * 2 + fl
            for qc in range(QC):
                ps = ps_h.tile([P, NCH], F32, name="hmm", tag="hmm")
                for kt in range(DT):
                    nc.tensor.matmul(
                        ps[:],
                        w1c[:, kt, fl * P:(fl + 1) * P],
                        n1bf[kt][:, qc * NCH:(qc + 1) * NCH],
                        start=(kt == 0), stop=(kt == DT - 1),
                    )
                nc.scalar.activation(
                    h_sb[ft][:, qc * NCH:(qc + 1) * NCH], ps[:], AF.Gelu,
                )

    es_w1.close()

    es_w2 = ExitStack()
    ps_f = es_w2.enter_context(tc.tile_pool(name="ps_f", bufs=2, space="PSUM"))
    w2s = es_w2.enter_context(tc.tile_pool(name="w2s", bufs=2, side="right"))
    n2pre = [prep.tile([P, Q], F32, name=f"n2pre{i}", tag=f"pre{i}")
             for i in range(DT)]

    W2CH = 256
    for oc in range(D // W2CH):           # 3 chunks of 256 output features
        w2c = w2s.tile([P, FT, W2CH], BF16, name="w2c", tag="w2c")
        nc.sync.dma_start(
            out=w2c[:],
            in_=w2T_r[:, :, oc * W2CH:(oc + 1) * W2CH].rearrange("t p m -> p t m"),
        )
        if oc == 0:
            tps = ps_f.tile([1, 8], F32, name="tch2", tag="fmm")
            nc.tensor.matmul(tps[:], w2c[0:1, 0, 0:1], w2c[0:1, 0, 0:8],
                             start=True, stop=True)
        for ol in range(W2CH // P):
            ot = oc * 2 + ol
            for qc in range(QC):
                ps = ps_f.tile([P, NCH], F32, name="fmm", tag="fmm")
                for ft in range(FT):
                    nc.tensor.matmul(
                        ps[:],
                        w2c[:, ft, ol * P:(ol + 1) * P],
                        h_sb[ft][:, qc * NCH:(qc + 1) * NCH],
                        start=(ft == 0), stop=(ft == FT - 1),
                    )
                nc.vector.tensor_add(
                    n2pre[ot][:, qc * NCH:(qc + 1) * NCH], ps[:],
                    n1f[ot][:, qc * NCH:(qc + 1) * NCH],
                )

    es_w2.close()
    es_h.close()
    es_n1.close()

    # ---------------- phase 5: LN2 + store ----------------
    _otiles = {}

    def ln2_dst(ot, qc):
        o = outp.tile([P, NCH], F32, name="o", tag="o", bufs=2)
        _otiles[(ot, qc)] = o
        return o[:]

    def ln2_out(ot, qc, dst):
        nc.sync.dma_start(out=outT_r[ot][:, qc * NCH:(qc + 1) * NCH],
                          in_=_otiles[(ot, qc)][:])

    layer_norm(n2pre, ps_stats, ps_bcln, ln2_dst, ln2_out)

    es_ln.close()
    es_pre.close()
    es.close()


class _EncBacc(bacc.Bacc):
    """Bacc whose activation-table pass never picks the exp-only set, so Exp
    and Ln co-reside in natural_log_exp_and_others (no table thrashing)."""

    def insert_act_table_loads(self):
        import bass_rust as _bass_rust
        from concourse.hw_specs import get_activation_tables
        has_activation = any(
            isinstance(i, mybir.InstActivation)
            for b in self.main_func.blocks
            for i in b.instructions
        )
        if not has_activation:
            return
        AFT = mybir.ActivationFunctionType
        tables = []
        for name, funcs in get_activation_tables(self.m.arch).items():
            if name in ("exp_and_others", "exp_and_friends"):
                funcs = funcs - {AFT.Exp}
            tables.append((name, funcs))
        _bass_rust.insert_act_table_loads(self, tables)


def build():
    nc = _EncBacc("TRN2", target_bir_lowering=False, debug=False)
    t = {}
    t["xT"] = nc.dram_tensor("xT", [D, S], F32, kind="ExternalInput").ap()
    for name, shape in [
        ("wqT", [D, D]), ("wkT", [D, D]), ("wvT", [D, D]), ("woT", [D, D]),
        ("w1T", [D, F]), ("w2T", [F, D]),
    ]:
        t[name] = nc.dram_tensor(name, shape, BF16, kind="ExternalInput").ap()
    t["maskc"] = nc.dram_tensor("maskc", [P, ST], F32, kind="ExternalInput").ap()
    t["lng"] = nc.dram_tensor("lng", [P, DT], F32, kind="ExternalInput").ap()
    t["lnb"] = nc.dram_tensor("lnb", [P, DT], F32, kind="ExternalInput").ap()
    t["outT"] = nc.dram_tensor("outT", [D, Q], F32, kind="ExternalOutput").ap()

    with tile.TileContext(nc) as tc:
        _emit(nc, tc, t)
    nc.compile()
    return nc


_NC_CACHE = None


def _get_nc():
    global _NC_CACHE
    if _NC_CACHE is None:
        _NC_CACHE = build()
    return _NC_CACHE


def make_in_maps(hidden_state, key_padding_mask, Wq, Wk, Wv, Wo, W1, W2, ln_g, ln_b):
    x = np.asarray(hidden_state, dtype=np.float32)
    mask = np.asarray(key_padding_mask)
    bf = ml_dtypes.bfloat16

    shared = {
        "wqT": (np.asarray(Wq, np.float32).T * 0.125).astype(bf),
        "wkT": np.asarray(Wk, np.float32).T.astype(bf),
        "wvT": np.asarray(Wv, np.float32).T.astype(bf),
        "woT": np.asarray(Wo, np.float32).T.astype(bf),
        "w1T": np.asarray(W1, np.float32).T.astype(bf),
        "w2T": np.asarray(W2, np.float32).T.astype(bf),
        "lng": np.ascontiguousarray(
            np.asarray(ln_g, np.float32).reshape(DT, P).T),
        "lnb": np.ascontiguousarray(
            np.asarray(ln_b, np.float32).reshape(DT, P).T),
    }
    shared = {k: np.ascontiguousarray(v) for k, v in shared.items()}

    maskadd = np.where(mask == 0, np.float32(MASK_NEG), np.float32(0.0))

    in_maps = []
    for c in range(N_CORES):
        b, half = c // 2, c % 2
        perm = np.r_[np.arange(half * Q, (half + 1) * Q),
                     np.arange((1 - half) * Q, (2 - half) * Q)]
        m = dict(shared)
        m["xT"] = np.ascontiguousarray(x[b].T[:, perm])
        m["maskc"] = np.ascontiguousarray(
            maskadd[b][perm].astype(np.float32).reshape(ST, P).T)
        in_maps.append(m)
    return in_maps


def gather(results, N=4):
    out = np.empty((N, S, D), dtype=np.float32)
    for c in range(N_CORES):
        b, half = c // 2, c % 2
        out[b, half * Q:(half + 1) * Q, :] = results[c]["outT"].T
    return out


def _install_ntff_hook():
    """Provide antenv.axon_hooks (absent in this image) so that
    run_bass_kernel_spmd(trace=True) can profile via the axon .so."""
    import types
    import antenv
    if getattr(antenv, "axon_hooks", None) is not None:
        return
    from trn_agent_boot.trn_boot import _ntff_profile_via_ctypes
    hook = _ntff_profile_via_ctypes("/opt/axon/libaxon_pjrt.so")
    mod = types.ModuleType("antenv.axon_hooks")
    mod._hook = hook
    mod.get_axon_ntff_profile_hook = lambda: mod._hook
    mod.set_axon_ntff_profile_hook = lambda h: setattr(mod, "_hook", h)
    sys.modules["antenv.axon_hooks"] = mod
    antenv.axon_hooks = mod


def kernel(hidden_state, key_padding_mask, Wq, Wk, Wv, Wo, W1, W2, ln_g, ln_b,
           trace=False):
    from concourse.bass_utils import run_bass_kernel_spmd

    if trace:
        _install_ntff_hook()

    nc = _get_nc()
    in_maps = make_in_maps(hidden_state, key_padding_mask,
                           Wq, Wk, Wv, Wo, W1, W2, ln_g, ln_b)
    res = run_bass_kernel_spmd(nc, in_maps, core_ids=list(range(N_CORES)),
                               trace=trace)
    out = gather(res.results)
    if trace:
        return out, res
    return out
